# revision 10
# baseline (speedup 1.0000x reference)
"""Trainium2 Bass kernel for nn_BimodalAttention.

Reference computation (B=128, L=512, D=256, T=64, G=8):
  aco_p/vis_p = group-mean pool (8->1) along L            [B,T,D]
  c_att = sigmoid(cw0*aco_p + cw1*vis_p + cb)             [B,T,D]
  hw    = 0.5*(aco_p+vis_p)
  h_att = sigmoid(hw.mean(D) @ Wh.T + bh)                 [B,T]
  w_att = sigmoid(hw.mean(T) @ Ww.T + bw)                 [B,D]
  scale = (h_att[map] + w_att + c_att[map]) / 3           [B,L,D]
  out   = where(IS_BAG==1, in*scale, in)  for both modalities

Sharding: pure data parallel, 16 batches per core on 8 cores.

Per-core layout: batch slice [512,256] as [128p, (n=4, d=256)] with
l = p*4 + n (4KB contiguous DRAM per partition), so pooled frame
t = l//8 = p//2 independent of n.  The pooling selector has duplicated
rows (psel2[p,q] = 1/8 iff p//2 == q//2) so pooled tensors live on all
128 partitions -- no pooled->full broadcast is ever needed.  Both
modalities pool in one PSUM-accumulated f32r matmul per n-block straight
from the fp32 input (f32r moving is 1 cyc/row at N>=256): no bf16
conversion passes.

DMA: acoustic loads+stores ride qSyncDynamicHW (sync engine), visual
loads+stores + consts ride qActDynamicHW (scalar engine) -- ~16.5 MB per
HWDGE FIFO, so neither queue head-of-line blocks the other modality.
Transfers are 1 MB (two batches per DMA).

Everything data-independent (selector, dup-transposed Wh, Ww^T, dup'd
biases, conv-scalar broadcast, IS_BAG mask layout) is precomputed on the
host; device setup is just ~420 KB of const DMAs and two bf16 casts.
"""

import sys
from contextlib import ExitStack

import numpy as np

sys.path.insert(0, "/opt/trn_rl_repo")

import concourse.bass as bass  # noqa: E402
import concourse.tile as tile  # noqa: E402
from concourse import bacc, mybir  # noqa: E402
from concourse.bass_utils import run_bass_kernel_spmd  # noqa: E402

B, L, D = 128, 512, 256
T = 64
G = L // T          # 8
NCORES = 8
BPC = B // NCORES   # 16 batches per core
NB = L // 128       # 4 n-blocks
SB = BPC // 2       # 8 super-batches of 2
F32 = mybir.dt.float32
F32R = mybir.dt.float32r
BF16 = mybir.dt.bfloat16
AF = mybir.ActivationFunctionType
OP = mybir.AluOpType


def bimodal_body(ctx: ExitStack, tc: "tile.TileContext", ins: dict, outs: dict):
    nc = tc.nc
    aco, vis = ins["aco"], ins["vis"]
    psel2, wh3, wwt = ins["psel2"], ins["wh3"], ins["wwt"]
    bh128, bw256, cvec_d, mask_d = (
        ins["bh128"], ins["bw256"], ins["cvec"], ins["maskc"])
    aco_o, vis_o = outs["aco_o"], outs["vis_o"]

    io_in = ctx.enter_context(tc.tile_pool(name="io_in", bufs=4))
    io_out = ctx.enter_context(tc.tile_pool(name="io_out", bufs=3))
    fpool = ctx.enter_context(tc.tile_pool(name="fpool", bufs=3))
    small = ctx.enter_context(tc.tile_pool(name="small", bufs=4))
    const = ctx.enter_context(tc.tile_pool(name="const", bufs=1))
    pp = ctx.enter_context(tc.tile_pool(name="pp", bufs=2, space="PSUM"))
    dp = ctx.enter_context(tc.tile_pool(name="dp", bufs=2, space="PSUM"))
    sp = ctx.enter_context(tc.tile_pool(name="sp", bufs=2, space="PSUM"))

    # --- consts first on the scalar queue (tiny, unblock the pipeline) --
    psel_sb = const.tile([128, 128], F32R)
    nc.scalar.dma_start(psel_sb[:], psel2)
    cvec = const.tile([128, 4], F32)
    nc.scalar.dma_start(cvec[:], cvec_d)
    mask_sb = const.tile([128, 64], F32)
    nc.scalar.dma_start(mask_sb[:], mask_d)
    wh3_f = const.tile([128, 128], F32)
    nc.scalar.dma_start(wh3_f[:], wh3)
    wwt_f = const.tile([128, 512], F32)
    nc.scalar.dma_start(
        wwt_f[:].rearrange("p (h d) -> p h d", h=2),
        wwt.rearrange("(h p) d -> p h d", h=2))
    bh_sb = const.tile([128, 1], F32)
    nc.scalar.dma_start(bh_sb[:], bh128[:, None])
    bw_row = const.tile([1, 256], F32)
    nc.scalar.dma_start(bw_row[:], bw256[None, :])

    # --- big loads: modality-split across the two HWDGE queues ----------
    def load_super(sb):
        t_in = io_in.tile([128, 2 * 2 * 4 * 256], F32R, tag="in2")
        v = t_in[:].rearrange("p (bb m n d) -> p bb m n d", bb=2, m=2, d=256)
        nc.sync.dma_start(
            v[:, :, 0],
            aco[2 * sb:2 * sb + 2].rearrange("bb (p n) d -> p bb n d", n=NB))
        nc.scalar.dma_start(
            v[:, :, 1],
            vis[2 * sb:2 * sb + 2].rearrange("bb (p n) d -> p bb n d", n=NB))
        return t_in

    in_tiles = {}
    for sbi in range(3):
        in_tiles[sbi] = load_super(sbi)

    # one-time bf16 casts + tiny consts (no DMA, no PE)
    wh3_b = const.tile([128, 128], BF16)
    nc.vector.tensor_copy(wh3_b[:], wh3_f[:])
    wwt_b = const.tile([128, 512], BF16)
    nc.vector.tensor_copy(wwt_b[:], wwt_f[:])
    ones_col = const.tile([128, 1], BF16)
    nc.gpsimd.memset(ones_col[:], 1.0)
    third_f = const.tile([1, 128], F32)
    nc.gpsimd.memset(third_f[:], 1.0 / 3.0)
    third_row = const.tile([1, 128], F32R)
    nc.vector.tensor_copy(third_row[:], third_f[:])

    def stage_pool(b):
        """PE pooling: both modalities, accumulated over n-blocks."""
        sb, bb = divmod(b, 2)
        t_in = in_tiles[sb]
        iv = t_in[:].rearrange("p (bb m n d) -> p bb m n d", bb=2, m=2, d=256)
        pool_ps = pp.tile([128, 512], F32, tag="pp")
        for n in range(NB):
            rhs = iv[:, bb, :, n, :]
            nc.tensor.matmul(pool_ps[:].rearrange("p (m d) -> p m d", m=2),
                             psel_sb[:], rhs, start=(n == 0), stop=(n == NB - 1))
        return {"pool": pool_ps, "in": t_in}

    def stage_stats(b, st):
        pool_ps = st["pool"]
        pa = pool_ps[:, 0:256]
        pv = pool_ps[:, 256:512]
        # c_att = sigmoid(cw0*pa + cw1*pv + cb)
        s1 = small.tile([128, 256], F32, tag="s1")
        nc.vector.tensor_scalar(s1[:], pv, cvec[:, 1:2], cvec[:, 2:3],
                                op0=OP.mult, op1=OP.add)
        c_pre = small.tile([128, 256], F32, tag="c_pre")
        nc.vector.scalar_tensor_tensor(c_pre[:], pa, cvec[:, 0:1], s1[:],
                                       op0=OP.mult, op1=OP.add)
        c_att = small.tile([128, 256], F32, tag="c_att")
        nc.scalar.activation(c_att[:], c_pre[:], AF.Sigmoid)
        # hw = pa + pv (=2*hw_ref), row sums -> hmean
        pv_sb = small.tile([128, 256], BF16, tag="pv_sb")
        nc.vector.tensor_scalar(pv_sb[:], pv, 1.0, None, op0=OP.mult)
        hw_sb = small.tile([128, 256], BF16, tag="hw")
        hmean = small.tile([128, 1], F32, tag="hmean")
        nc.vector.scalar_tensor_tensor(hw_sb[:], pa, 0.0, pv_sb[:],
                                       op0=OP.add, op1=OP.add,
                                       accum_out=hmean[:])
        hmean_b = small.tile([128, 1], BF16, tag="hmean_b")
        nc.vector.tensor_copy(hmean_b[:], hmean[:])
        # d-sums of hw into PSUM columns (hw as stationary, N=1)
        dsum = dp.tile([128, 4], F32, tag="dp")
        nc.tensor.matmul(dsum[:, 0:1], hw_sb[:, 0:128], ones_col[:],
                         start=True, stop=True)
        nc.tensor.matmul(dsum[:, 1:2], hw_sb[:, 128:256], ones_col[:],
                         start=True, stop=True)
        # h_att = sigmoid(hsum/512 + bh)
        nc.tensor.matmul(dsum[:, 2:3], wh3_b[:], hmean_b[:],
                         start=True, stop=True)
        h_att = small.tile([128, 1], F32, tag="h_att")
        nc.scalar.activation(h_att[:], dsum[:, 2:3], AF.Sigmoid,
                             bias=bh_sb[:], scale=1.0 / 512.0)
        hm1 = small.tile([128, 1], F32, tag="hm1")
        nc.vector.tensor_scalar(hm1[:], h_att[:], 1.0 / 3.0, -1.0,
                                op0=OP.mult, op1=OP.add)
        mask4 = mask_sb[:, b:b + 1]
        mask4 = bass.AP(mask4.tensor, mask4.offset, [mask4.ap[0], [16, 4]])
        b_all = small.tile([128, 4], F32, tag="b_all")
        nc.vector.tensor_scalar(b_all[:], mask4, hm1[:], 1.0,
                                op0=OP.mult, op1=OP.add)
        # w_att = sigmoid((wsum @ wwT + 256*bw)/256)
        wsum = small.tile([128, 2], BF16, tag="wsum")
        nc.vector.tensor_scalar(wsum[:], dsum[:, 0:2], 1.0, None, op0=OP.mult)
        stat_ps = sp.tile([128, 512], F32, tag="sp")
        nc.tensor.matmul(stat_ps[0:1, 256:512], wsum[:, 0:1], wwt_b[:, 0:256],
                         start=True, stop=False)
        nc.tensor.matmul(stat_ps[0:1, 256:512], wsum[:, 1:2], wwt_b[:, 256:512],
                         start=False, stop=True)
        wrow = small.tile([1, 256], F32, tag="wrow")
        nc.vector.tensor_tensor(wrow[:], stat_ps[0:1, 256:512], bw_row[:],
                                op=OP.add)
        w_sig = small.tile([1, 256], F32R, tag="w_sig")
        nc.scalar.activation(w_sig[:], wrow[:], AF.Sigmoid, scale=1.0 / 256.0)
        # broadcast w_sig/3 to all partitions via K=1 matmul
        nc.tensor.matmul(stat_ps[:, 0:256], third_row[:], w_sig[:],
                         start=True, stop=True)
        # Sb3 = c_att/3 + w/3
        sb3 = small.tile([128, 256], F32, tag="sb3")
        nc.vector.scalar_tensor_tensor(sb3[:], c_att[:], 1.0 / 3.0,
                                       stat_ps[:, 0:256],
                                       op0=OP.mult, op1=OP.add)
        st.update(sb3=sb3, b_all=b_all)
        return st

    pending = []

    def emit_store(sb, t_out):
        ov = t_out[:].rearrange("p (bb m n d) -> p bb m n d", bb=2, m=2, d=256)
        nc.sync.dma_start(
            aco_o[2 * sb:2 * sb + 2].rearrange("bb (p n) d -> p bb n d", n=NB),
            ov[:, :, 0])
        nc.scalar.dma_start(
            vis_o[2 * sb:2 * sb + 2].rearrange("bb (p n) d -> p bb n d", n=NB),
            ov[:, :, 1])

    out_tiles = {}

    def stage_apply(b, st):
        sb, bb = divmod(b, 2)
        sb3, b_all = st["sb3"], st["b_all"]
        t_in = st["in"]
        if bb == 0:
            out_tiles[sb] = io_out.tile([128, 2 * 2 * 4 * 256], F32, tag="out2",
                                        name="t_out")
        t_out = out_tiles[sb]
        # F_n = mask_n * Sb3 + (mask_n*(h/3 - 1) + 1)   [128, (n,d)]  bf16
        f_t = fpool.tile([128, 1024], BF16, tag="f")
        for n in range(4):
            col = n * 16 + b
            blk = slice(n * 256, (n + 1) * 256)
            if n < 2:
                nc.scalar.activation(f_t[:, blk], sb3[:], AF.Identity,
                                     bias=b_all[:, n:n + 1],
                                     scale=mask_sb[:, col:col + 1])
            elif n == 2:
                nc.vector.tensor_scalar(f_t[:, blk], sb3[:],
                                        mask_sb[:, col:col + 1],
                                        b_all[:, n:n + 1],
                                        op0=OP.mult, op1=OP.add)
            else:
                nc.gpsimd.tensor_scalar(f_t[:, blk], sb3[:],
                                        mask_sb[:, col:col + 1],
                                        b_all[:, n:n + 1],
                                        op0=OP.mult, op1=OP.add)
        # out = in * F ; acoustic on DVE, visual on GpSimd
        a0 = bb * 2048
        v0 = bb * 2048 + 1024
        nc.vector.tensor_tensor(t_out[:, a0:a0 + 1024], t_in[:, a0:a0 + 1024],
                                f_t[:], op=OP.mult)
        nc.gpsimd.tensor_tensor(t_out[:, v0:v0 + 1024], t_in[:, v0:v0 + 1024],
                                f_t[:], op=OP.mult)
        if bb == 1:
            pending.append((sb, t_out))
            if len(pending) > 1:
                emit_store(*pending.pop(0))

    # software-pipelined main loop, 3 skewed stages
    states = {}
    for b in range(BPC):
        sb, bb = divmod(b, 2)
        if bb == 0 and sb + 3 < SB:
            in_tiles[sb + 3] = load_super(sb + 3)
        states[b] = stage_pool(b)
        if b >= 1:
            stage_stats(b - 1, states[b - 1])
        if b >= 2:
            stage_apply(b - 2, states[b - 2])
            del states[b - 2]
    stage_stats(BPC - 1, states[BPC - 1])
    stage_apply(BPC - 2, states[BPC - 2])
    stage_apply(BPC - 1, states[BPC - 1])
    for item in pending:
        emit_store(*item)


def build_nc():
    nc = bacc.Bacc("TRN2", target_bir_lowering=False, debug=False,
                   num_devices=NCORES)
    ins = {
        "aco": nc.dram_tensor("aco", [BPC, L, D], F32R, kind="ExternalInput").ap(),
        "vis": nc.dram_tensor("vis", [BPC, L, D], F32R, kind="ExternalInput").ap(),
        "psel2": nc.dram_tensor("psel2", [128, 128], F32R, kind="ExternalInput").ap(),
        "wh3": nc.dram_tensor("wh3", [128, 128], F32, kind="ExternalInput").ap(),
        "wwt": nc.dram_tensor("wwt", [D, D], F32, kind="ExternalInput").ap(),
        "bh128": nc.dram_tensor("bh128", [128], F32, kind="ExternalInput").ap(),
        "bw256": nc.dram_tensor("bw256", [D], F32, kind="ExternalInput").ap(),
        "cvec": nc.dram_tensor("cvec", [128, 4], F32, kind="ExternalInput").ap(),
        "maskc": nc.dram_tensor("maskc", [128, 64], F32, kind="ExternalInput").ap(),
    }
    outs = {
        "aco_o": nc.dram_tensor("aco_o", [BPC, L, D], F32, kind="ExternalOutput").ap(),
        "vis_o": nc.dram_tensor("vis_o", [BPC, L, D], F32, kind="ExternalOutput").ap(),
    }
    with tile.TileContext(nc) as tc:
        with ExitStack() as ctx:
            bimodal_body(ctx, tc, ins, outs)
    nc.compile()
    return nc


_NC_CACHE = None


def _get_nc():
    global _NC_CACHE
    if _NC_CACHE is None:
        _NC_CACHE = build_nc()
    return _NC_CACHE


def _host_consts(inputs):
    wh = np.asarray(inputs["Wh"], dtype=np.float32)
    ww = np.asarray(inputs["Ww"], dtype=np.float32)
    bh = np.asarray(inputs["bh"], dtype=np.float32)
    bw = np.asarray(inputs["bw"], dtype=np.float32)
    cw = np.asarray(inputs["conv_w"], dtype=np.float32)
    cb = np.asarray(inputs["conv_b"], dtype=np.float32)
    q = np.arange(128)
    psel2 = ((q[:, None] // 2) == (q[None, :] // 2)).astype(np.float32) / G
    wh3 = np.repeat(np.repeat(wh.T, 2, axis=0), 2, axis=1).astype(np.float32) / 2.0
    cvec = np.tile(np.concatenate([cw, cb, np.zeros(1, np.float32)]), (128, 1))
    return {
        "psel2": np.ascontiguousarray(psel2),
        "wh3": np.ascontiguousarray(wh3),
        "wwt": np.ascontiguousarray(ww.T),
        "bh128": np.ascontiguousarray(np.repeat(bh, 2)),
        "bw256": np.ascontiguousarray(256.0 * bw),
        "cvec": np.ascontiguousarray(cvec.astype(np.float32)),
    }


def _run(inputs: dict, trace: bool = False, tmpdir=None):
    nc = _get_nc()
    acoustic = np.ascontiguousarray(np.asarray(inputs["acoustic_seq"], dtype=np.float32))
    visual = np.ascontiguousarray(np.asarray(inputs["visual_seq"], dtype=np.float32))
    isbag = np.asarray(inputs["IS_BAG_list"], dtype=np.int32)
    shared = _host_consts(inputs)
    in_maps = []
    for m in range(NCORES):
        sl = slice(m * BPC, (m + 1) * BPC)
        # mask[p, n*16+b] = 1.0 iff IS_BAG[b, p*4+n] == 1
        bs = isbag[sl].reshape(BPC, 128, NB)
        maskc = (bs == 1).astype(np.float32).transpose(1, 2, 0).reshape(128, 64)
        in_maps.append({
            "aco": acoustic[sl], "vis": visual[sl],
            "maskc": np.ascontiguousarray(maskc), **shared})
    res = run_bass_kernel_spmd(nc, in_maps, core_ids=list(range(NCORES)),
                               trace=trace, tmpdir=tmpdir)
    aco_out = np.concatenate([res.results[m]["aco_o"] for m in range(NCORES)], axis=0)
    vis_out = np.concatenate([res.results[m]["vis_o"] for m in range(NCORES)], axis=0)
    return (aco_out, vis_out), res


def kernel(**inputs) -> np.ndarray:
    (aco_out, vis_out), _ = _run(inputs)
    return aco_out, vis_out
